# revision 65
# baseline (speedup 1.0000x reference)
"""CAM-module (complex channel-attention) Bass kernel for Trainium2.

Problem: x (2, 8, 512, 4, 32, 32) fp32 -> out same shape.
  qr, qi = x[0].reshape(B,C,N), x[1].reshape(B,C,N)   C=512, N=4096
  er = qr qr^T - qi qi^T ; ei = qr qi^T + qi qr^T     (B, C, C)
  F  = (rowmax(er)-er)^2 + (rowmax(ei)-ei)^2
  att = softmax_row(F)                                 (ultra-sharp)
  out = stack(g*att@qr + qr_in, g*att@qi + qi_in)

Sharding: data-parallel over batch B across 8 NeuronCores (core b = batch b).

Numerics: fp16 hi/lo split q = A + B (A = f16(q), B = q - A).  Hi products
run fp16 (exact in PE's fp22 path); lo corrections only contribute at
2^-11 relative scale, so they need just ~2^-5 relative accuracy: they run
in scaled e4m3 fp8 (a8 = f8(q), b8 = f8(B*2^13)) using DoubleRow perf
mode, which contracts 2 k-chunks per instruction at 0.5 cyc/row:
  er = [Ar Ar^T - Ai Ai^T]_tri  + M + M^T,  M = (a8r b8r^T - a8i b8i^T)/2^13
  ei = S + S^T,  S = Ar Ai^T + (a8r b8i^T + b8r a8i^T)/2^13
The pure-hi Gram terms (ArAr^T, AiAi^T) are symmetric: only the lower
block-triangle is computed; upper blocks mirror via PE transposes.
Measured numerics vs the jax fp32 reference: rel err ~2e-4 (gate 2e-2),
0 argmax flips.

Phase 2 (att @ q) runs fp16.  During phase 0 the fp16 hi of x is also
written back to a DRAM scratch (xh) in original layout; phase 2 streams
xh (8 MiB instead of 16 MiB fp32) and uses it for both the matmul rhs
and the +x residual (residual err 2^-11 |x| ~ 2.4e-3 abs, gate ~0.1).

Schedule notes:
 - input loaded in column-quarters; pass-0 hi-Gram interleaves with the
   PE input transposes; lo (DoubleRow) matmuls for pass 0 run right after.
 - per m-pass: hi products (tri u1/u2 + full S_hi), evac-u, lo DR
   products, evac-lo (scaled 2^-13 via STT), then band assembly (tri er/ei
   blocks + transpose-mirrors) emitted AFTER the next pass's hi matmuls so
   the PE never waits on the DVE/ACT evacuation.
 - phase-2 slab DMA is emitted before the softmax tail so it overlaps;
 - discarded keep-warm matmuls are pinned (via explicit deps) into the
   softmax tail and phase-2 slab stalls so no PE-idle gap exceeds the
   ~3.4us HAM window.
"""
import sys, os
sys.path.insert(0, '/opt/trn_rl_repo')

import numpy as np
from contextlib import ExitStack

import concourse.bass as bass
import concourse.mybir as mybir
import concourse.tile as tile
from concourse import bacc
from concourse.bass_utils import run_bass_kernel_spmd
from concourse.masks import make_identity
from concourse.tile import add_dep_helper

F32 = mybir.dt.float32
F16 = mybir.dt.float16
F8 = mybir.dt.float8e4
AX = mybir.AxisListType
AF = mybir.ActivationFunctionType
OP = mybir.AluOpType
DR = mybir.MatmulPerfMode.DoubleRow

C = 512          # channels
N = 4096         # spatial (4*32*32)
NK = N // 128    # 32 n-chunks (contraction)
NP = NK // 2     # 16 DoubleRow k-pairs
MC = C // 128    # 4 c-chunks
NQ = 4           # column quarters for load/transpose pipeline
KQ = NK // NQ    # 8 n-chunks per quarter
NJ = N // 512    # phase-2 column blocks
SC = 8192.0      # 2^13 lo-operand scale
ISC = 1.0 / SC


def _kview(t):
    return t[:].rearrange("p (k c) -> p k c", c=C)


def _gram_hi(nc, accs, arT, aiT, m, k):
    """Hi fp16 products for (m-pass, k-chunk): tri u1/u2 + full S_hi.
    Grouped by lhsT (ar_m: u1, sh / ai_m: u2) for weight reuse."""
    u1, u2, sh = accs
    st, sp = (k == 0), (k == NK - 1)
    lo = k * C
    w = (m + 1) * 128
    ar_m = arT[:, lo + m * 128: lo + (m + 1) * 128]
    ai_m = aiT[:, lo + m * 128: lo + (m + 1) * 128]
    nc.tensor.matmul(u1[:, 0:w], ar_m, arT[:, lo:lo + w], start=st, stop=sp)
    nc.tensor.matmul(sh[:], ar_m, aiT[:, lo:lo + C], start=st, stop=sp)
    nc.tensor.matmul(u2[:, 0:w], ai_m, aiT[:, lo:lo + w], start=st, stop=sp)


def _gram_lo(nc, accs, a8r, a8i, b8r, b8i, m, kp):
    """Lo scaled-e4m3 DoubleRow products for (m-pass, k-pair kp)."""
    m1, m2, slo = accs
    st, sp = (kp == 0), (kp == NP - 1)
    mb = m * 128

    def pv(t, c0, cw):
        return _kview(t)[:, 2 * kp:2 * kp + 2, c0:c0 + cw]

    nc.tensor.matmul(m1[:], pv(a8r, mb, 128), pv(b8r, 0, C),
                     start=st, stop=sp, perf_mode=DR)
    nc.tensor.matmul(slo[:], pv(a8r, mb, 128), pv(b8i, 0, C),
                     start=st, stop=False, perf_mode=DR)
    nc.tensor.matmul(m2[:], pv(a8i, mb, 128), pv(b8i, 0, C),
                     start=st, stop=sp, perf_mode=DR)
    nc.tensor.matmul(slo[:], pv(b8r, mb, 128), pv(a8i, 0, C),
                     start=False, stop=sp, perf_mode=DR)


def build_kernel():
    nc = bacc.Bacc("TRN2", target_bir_lowering=False, debug=False,
                   enable_asserts=False)
    x_dram = nc.dram_tensor("x", (2, C, N), F32, kind="ExternalInput").ap()
    g_dram = nc.dram_tensor("gamma", (1,), F32, kind="ExternalInput").ap()
    y_dram = nc.dram_tensor("y", (2, C, N), F32, kind="ExternalOutput").ap()
    xh_dram = nc.dram_tensor("xh", (2, C, N), F16, kind="Internal").ap()
    # [128p, 4d, n] strided views for single-DMA slab loads / y stores
    xh_v = [xh_dram[ti].rearrange("(d p) n -> p d n", p=128) for ti in range(2)]
    y_v = [y_dram[ti].rearrange("(d p) n -> p d n", p=128) for ti in range(2)]

    with tile.TileContext(nc) as tc, ExitStack() as ctx:
        const = ctx.enter_context(tc.tile_pool(name="const", bufs=1))
        small = ctx.enter_context(tc.tile_pool(name="small", bufs=24))
        sqf = ctx.enter_context(tc.tile_pool(name="sqf", bufs=1))
        sqb = ctx.enter_context(tc.tile_pool(name="sqb", bufs=1))

        ident32 = const.tile([128, 128], F32, tag="id32")
        make_identity(nc, ident32[:])
        ident16 = const.tile([128, 128], F16, tag="id16")
        make_identity(nc, ident16[:])
        ident8 = const.tile([128, 128], F8, tag="id8")
        make_identity(nc, ident8[:])
        ones16 = const.tile([128, 512], F16, tag="ones16")
        nc.gpsimd.memset(ones16[:], 1.0)
        g_bc = const.tile([128, 1], F32, tag="gbc")
        # gamma is only read by the phase-2 epilogue; use SWDGE so these 128
        # tiny broadcast descriptors don't delay the first input loads on
        # the HWDGE queue.
        nc.gpsimd.dma_start(g_bc[:], g_dram[None, :].partition_broadcast(128))

        # persistent [512,512] matrices as [128, 4*512] (row-chunk r at cols r*512)
        er_sb = sqf.tile([128, MC * C], F32, tag="er")
        m12_sb = sqf.tile([128, MC * C], F32, tag="m12")
        s_sb = sqf.tile([128, MC * C], F32, tag="s")
        ei_sb = sqf.tile([128, MC * C], F32, tag="ei")
        attT_sb = sqb.tile([128, MC * C], F16, tag="attT")
        # running row-max of er/ei, accumulated band by band so the softmax
        # tail skips the big reductions
        runm = ctx.enter_context(tc.tile_pool(name="runm", bufs=8))
        run_mx = {(mat, r): runm.tile([128, 1], F32, name=f"runmx_{mat}_{r}")
                  for mat in (0, 1) for r in range(MC)}

        with tc.tile_pool(name="opsT", bufs=2) as opsT, \
             tc.tile_pool(name="ops8", bufs=4) as ops8:
            # transposed operands, [128, NK*512]; chunk k at cols k*512
            arT = opsT.tile([128, NK * C], F16, tag="opsT")
            aiT = opsT.tile([128, NK * C], F16, tag="opsT")
            a8r = ops8.tile([128, NK * C], F8, tag="ops8")
            a8i = ops8.tile([128, NK * C], F8, tag="ops8")
            b8r = ops8.tile([128, NK * C], F8, tag="ops8")
            b8i = ops8.tile([128, NK * C], F8, tag="ops8")
            hi8 = [(arT, a8r, b8r), (aiT, a8i, b8i)]

            with tc.tile_pool(name="acc0", bufs=3, space="PSUM") as acc0:
                hi_accs = {m: None for m in range(MC)}
                lo_accs = {m: None for m in range(MC)}
                hi_accs[0] = [acc0.tile([128, C], F32, tag="acc",
                                        name=f"hia0_{i}") for i in range(3)]

                # --- Phase 0 + pass-0 hi Gram, interleaved by half ----------
                # one [128,2048] load unit per (H, ti, j): 16 input DMAs
                # total (HWDGE descriptor generation is ~625ns per DMA), each
                # split into two [128,1024] transpose/cast pipeline steps.
                with tc.tile_pool(name="tpose", bufs=2, space="PSUM") as tpose, \
                     tc.tile_pool(name="stage", bufs=3, side="right") as stage, \
                     tc.tile_pool(name="a16p", bufs=2, side="right") as a16p, \
                     tc.tile_pool(name="lo16p", bufs=3, side="right") as lo16p:
                    KH = NK // 2        # 16 k-chunks per half
                    for H in range(2):
                        kh = H * KH
                        for ti, (aT, a8, b8) in enumerate(hi8):
                            for j in range(MC):     # c-chunk (rows)
                                q_t = stage.tile([128, 2048], F32, tag="stage")
                                nc.sync.dma_start(
                                    q_t[:], x_dram[ti, j * 128:(j + 1) * 128,
                                                   kh * 128:(kh + 16) * 128])
                                # fp16 hi in original layout -> DRAM scratch.
                                # Issued on the gpsimd SWDGE queue: on the SP
                                # queue its wait on the a16 cast head-of-line
                                # blocks the next input load.
                                a16 = a16p.tile([128, 2048], F16, tag="a16")
                                nc.gpsimd.tensor_copy(a16[:], q_t[:])
                                nc.gpsimd.dma_start(
                                    xh_dram[ti, j * 128:(j + 1) * 128,
                                            kh * 128:(kh + 16) * 128], a16[:])
                                cs = slice(j * 128, (j + 1) * 128)
                                for u in range(2):
                                    k0 = kh + u * 8
                                    pt = tpose.tile([128, 1024], F32, tag="pt")
                                    for t in range(8):
                                        nc.tensor.transpose(
                                            pt[:, t * 128:(t + 1) * 128],
                                            q_t[:, (u * 8 + t) * 128:
                                                (u * 8 + t + 1) * 128],
                                            ident32[:])
                                    aT_v = _kview(aT)[:, k0:k0 + 8, cs]
                                    a8_v = _kview(a8)[:, k0:k0 + 8, cs]
                                    b8_v = _kview(b8)[:, k0:k0 + 8, cs]
                                    pt_v = pt[:].rearrange(
                                        "p (t c) -> p t c", c=128)
                                    nc.scalar.copy(aT_v, pt_v)    # f32->f16 hi
                                    nc.scalar.copy(a8_v, pt_v)    # f32->f8 hi
                                    lo16 = lo16p.tile([128, 1024], F16,
                                                      tag="lo16")
                                    lo_v = lo16[:].rearrange(
                                        "p (t c) -> p t c", c=128)
                                    nc.vector.tensor_sub(lo_v, pt_v, aT_v)
                                    nc.vector.tensor_scalar_mul(b8_v, lo_v, SC)
                        # pass-0 hi Gram for this half's chunks
                        for k in range(kh, kh + KH):
                            _gram_hi(nc, hi_accs[0], arT, aiT, 0, k)



                # ------- m-passes: hi -> evac-u -> lo(DR) -> evac-lo ---------
                with tc.tile_pool(name="acc1", bufs=3, space="PSUM") as acc1, \
                     tc.tile_pool(name="symt", bufs=2, space="PSUM") as symt:
                    accp = {0: acc0, 1: acc1, 2: acc0, 3: acc1}
                    def evac_u(a):
                        """u1-u2 -> er row (partial), S_hi -> s row. Frees hi accs."""
                        u1, u2, sh = hi_accs[a]
                        w = (a + 1) * 128
                        er_a = er_sb[:, a * C: a * C + w]
                        nc.scalar.copy(er_a, u1[:, 0:w])
                        nc.vector.tensor_sub(er_a, er_a, u2[:, 0:w])
                        nc.scalar.copy(s_sb[:, a * C:(a + 1) * C], sh[:])

                    def evac_lo(a):
                        """Scaled lo products into m12 row / s row. Frees lo accs."""
                        m1, m2, slo = lo_accs[a]
                        m12_a = m12_sb[:, a * C:(a + 1) * C]
                        nc.scalar.activation(m12_a, m1[:], AF.Copy, scale=ISC)
                        nc.vector.scalar_tensor_tensor(
                            m12_a, m2[:], -ISC, m12_a, op0=OP.mult, op1=OP.add)
                        s_a = s_sb[:, a * C:(a + 1) * C]
                        nc.vector.scalar_tensor_tensor(
                            s_a, slo[:], ISC, s_a, op0=OP.mult, op1=OP.add)

                    def band(a):
                        """Tri er/ei blocks for row a + transpose-mirrors +
                        incremental row-max accumulation."""
                        w = (a + 1) * 128
                        er_a = er_sb[:, a * C: a * C + w]
                        ei_a = ei_sb[:, a * C: a * C + w]
                        m12_a = m12_sb[:, a * C: a * C + w]
                        s_a = s_sb[:, a * C: a * C + w]
                        nc.vector.tensor_add(er_a, er_a, m12_a)
                        # batched transposes of column-a blocks of m12 / s
                        for mat, src_sb, dst, base in (
                                (0, m12_sb, er_a, er_a),
                                (1, s_sb, ei_a, s_a)):
                            pt = symt.tile([128, 512], F32, tag="symt",
                                           name=f"symtb_{mat}_{a}")
                            for cb in range(a + 1):
                                nc.tensor.transpose(
                                    pt[:, cb * 128:(cb + 1) * 128],
                                    src_sb[:, cb * C + a * 128:
                                           cb * C + (a + 1) * 128],
                                    ident32[:])
                            nc.vector.tensor_add(dst, base, pt[:, 0:w])
                            # running row-max over the freshly completed row-a
                            # region (initializes run_mx[mat, a])
                            nc.vector.reduce_max(run_mx[(mat, a)][:], dst,
                                                 axis=AX.X)
                        # mirror finished tri blocks into the upper triangle
                        for cb in range(a):
                            for mat, m_sb in ((0, er_sb), (1, ei_sb)):
                                ptm = symt.tile([128, 128], F32, tag="symt",
                                                name=f"symtm_{mat}_{cb}_{a}")
                                nc.tensor.transpose(
                                    ptm[:],
                                    m_sb[:, a * C + cb * 128:
                                         a * C + (cb + 1) * 128],
                                    ident32[:])
                                mir = m_sb[:, cb * C + a * 128:
                                           cb * C + (a + 1) * 128]
                                # GPSIMD cannot read PSUM on hw: use ACT
                                nc.scalar.copy(mir, ptm[:])
                                pmx = small.tile([128, 1], F32, tag="small",
                                                 name=f"pmx_{mat}_{cb}_{a}")
                                nc.vector.reduce_max(pmx[:], mir, axis=AX.X)
                                nc.vector.tensor_tensor(
                                    run_mx[(mat, cb)][:], run_mx[(mat, cb)][:],
                                    pmx[:], op=OP.max)

                    def emit_lo(a):
                        lo_accs[a] = [accp[a].tile([128, C], F32, tag="acc",
                                                   name=f"loa{a}_{i}")
                                      for i in range(3)]
                        for kp in range(NP):
                            _gram_lo(nc, lo_accs[a], a8r, a8i, b8r, b8i, a, kp)

                    evac_u(0)
                    emit_lo(0)
                    evac_lo(0)
                    for m in range(1, MC):
                        hi_accs[m] = [accp[m].tile([128, C], F32, tag="acc",
                                                   name=f"hia{m}_{i}")
                                      for i in range(3)]
                        for k in range(NK):
                            _gram_hi(nc, hi_accs[m], arT, aiT, m, k)
                        # previous band emitted after this pass's hi matmuls:
                        # its PE transposes need evac_lo(m-1), which completes
                        # on DVE/ACT while the hi matmuls run.
                        band(m - 1)
                        evac_u(m)
                        emit_lo(m)
                        evac_lo(m)
                    band(MC - 1)

        # ------------- tail (softmax, attT) + Phase 2, overlapped -----------
        with tc.tile_pool(name="slab16a", bufs=8, side="right") as slab16a, \
             tc.tile_pool(name="slab16b", bufs=8) as slab16b, \
             tc.tile_pool(name="ysb", bufs=4) as ysbp, \
             tc.tile_pool(name="attp", bufs=2, space="PSUM") as attp, \
             tc.tile_pool(name="out", bufs=6, space="PSUM") as outp:

            # all slab DMA up front.  The right-side pool reuses the SBUF of
            # the (long closed) phase-0 staging pools, so its loads stream in
            # during the Gram's DMA-idle window; the left-side pool overlaps
            # the Gram operands and streams during the softmax tail.  Phase 2
            # is then mostly gated by the y writes.
            slabs, first_dma = {}, {}
            for j in range(NJ):
                pool = slab16a if j < NJ // 2 else slab16b
                for ti in range(2):
                    sl = pool.tile([128, MC, 512], F16, tag="sl16",
                                   name=f"sl_{j}_{ti}")
                    dma = nc.sync.dma_start(
                        sl[:, :, :], xh_v[ti][:, :, j * 512:(j + 1) * 512])
                    if ti == 0:
                        first_dma[j] = dma
                    slabs[(j, ti)] = sl

            # -------- softmax over squared magnitude, breadth-first ---------
            # row-maxes of er/ei were accumulated during the Gram passes
            # (run_mx); each stage is emitted for all m so the engines
            # pipeline across row-blocks instead of serializing each chain.
            with tc.tile_pool(name="smx", bufs=8) as smx, \
                 tc.tile_pool(name="attx", bufs=4) as attx:
                nmx, sq, fadds, rsums, atts = {}, {}, {}, {}, {}
                for m in range(MC):
                    for mat in (0, 1):
                        nmx[(mat, m)] = small.tile([128, 1], F32, tag="small",
                                                   name=f"nmx_{mat}_{m}")
                        nc.vector.tensor_scalar_mul(
                            nmx[(mat, m)][:], run_mx[(mat, m)][:], -1.0)
                for m in range(MC):
                    sq1 = smx.tile([128, C], F32, tag="smx", name=f"sq1_{m}")
                    sq2 = smx.tile([128, C], F32, tag="smx", name=f"sq2_{m}")
                    nc.scalar.activation(sq1[:], er_sb[:, m * C:(m + 1) * C],
                                         AF.Square, bias=nmx[(0, m)][:, 0:1])
                    nc.scalar.activation(sq2[:], ei_sb[:, m * C:(m + 1) * C],
                                         AF.Square, bias=nmx[(1, m)][:, 0:1])
                    sq[m] = (sq1, sq2)
                for m in range(MC):
                    fadds[m] = nc.vector.tensor_add(sq[m][0][:], sq[m][0][:],
                                                    sq[m][1][:])  # F
                # HAM keep-warm: discarded matmul pinned behind F via an
                # explicit dep so it fires mid-tail. PE-transposes don't count
                # as PE-busy for HAM; without this the first phase-2 matmuls
                # run throttled at 1.2 GHz.
                for m in (1, 3):
                    warm = outp.tile([128, 512], F32, tag="out",
                                     name=f"warm_{m}")
                    wmm = nc.tensor.matmul(warm[:], ident16[:], ones16[:],
                                           start=True, stop=True)
                    add_dep_helper(wmm.ins, fadds[m].ins, sync=True,
                                   reason="HAM keep-warm spacing")
                for m in range(MC):
                    nfm = small.tile([128, 1], F32, tag="small",
                                     name=f"nfm_{m}")
                    nc.vector.reduce_max(nfm[:], sq[m][0][:], axis=AX.X,
                                         negate=True)
                    rsum = small.tile([128, 1], F32, tag="small",
                                      name=f"rsum_{m}")
                    nc.scalar.activation(sq[m][1][:], sq[m][0][:], AF.Exp,
                                         bias=nfm[:, 0:1],
                                         accum_out=rsum[:, 0:1])
                    rsums[m] = rsum
                for m in range(MC):
                    rinv = small.tile([128, 1], F32, tag="small",
                                      name=f"rinv_{m}")
                    nc.vector.reciprocal(rinv[:], rsums[m][:])
                    att_t = attx.tile([128, C], F16, tag="attx",
                                      name=f"att_{m}")
                    nc.vector.tensor_scalar_mul(att_t[:], sq[m][1][:],
                                                rinv[:, 0:1])
                    atts[m] = att_t
                for m in range(MC):
                    # attT (m-major blocks: d at cols d*128)
                    pt = attp.tile([128, C], F16, tag="attt")
                    for d in range(MC):
                        nc.tensor.transpose(
                            pt[:, d * 128:(d + 1) * 128],
                            atts[m][:, d * 128:(d + 1) * 128],
                            ident16[:])
                    nc.scalar.copy(attT_sb[:, m * C:(m + 1) * C], pt[:])

            # ---------------- Phase 2 compute: gamma*(att@q) + x ------------
            for j in range(NJ):
                if j >= 2:
                    # HAM keep-warm across any slab-DMA stall at this boundary
                    warm = outp.tile([128, 512], F32, tag="out",
                                     name=f"warmj_{j}")
                    wmm = nc.tensor.matmul(warm[:], ident16[:], ones16[:],
                                           start=True, stop=True)
                    add_dep_helper(wmm.ins, first_dma[j].ins, sync=True,
                                   reason="HAM keep-warm phase-2")
                for ti in range(2):
                    sl = slabs[(j, ti)]
                    y_t = ysbp.tile([128, MC, 512], F32, tag="ysb")
                    for m in range(MC):
                        ops = outp.tile([128, 512], F32, tag="out")
                        for d in range(MC):
                            nc.tensor.matmul(
                                ops[:],
                                attT_sb[:, m * C + d * 128:
                                        m * C + (d + 1) * 128],
                                sl[:, d, :],
                                start=(d == 0), stop=(d == MC - 1))
                        # residual add on DVE (reads the PSUM accumulator;
                        # GPSIMD cannot access PSUM on hw)
                        nc.vector.scalar_tensor_tensor(
                            y_t[:, m, :], ops[:], g_bc[:, 0:1], sl[:, m, :],
                            op0=OP.mult, op1=OP.add)
                    # y stores issued from the ACT DGE queue (idle in phase
                    # 2) so descriptor generation for the read and write
                    # streams runs on two hardware rings instead of one.
                    nc.scalar.dma_start(
                        y_v[ti][:, :, j * 512:(j + 1) * 512], y_t[:, :, :])

    nc.compile()
    return nc


_NC_CACHE = None


def kernel(x: np.ndarray, gamma: np.ndarray) -> np.ndarray:
    global _NC_CACHE
    if _NC_CACHE is None:
        _NC_CACHE = build_kernel()
    nc = _NC_CACHE
    B = x.shape[1]
    x = np.ascontiguousarray(x, dtype=np.float32)
    in_maps = [{"x": np.ascontiguousarray(x[:, b]).reshape(2, C, N),
                "gamma": np.ascontiguousarray(gamma, dtype=np.float32)}
               for b in range(B)]
    res = run_bass_kernel_spmd(nc, in_maps, core_ids=list(range(B)))
    y = np.stack([res.results[b]["y"] for b in range(B)], axis=1)
    return y.reshape(x.shape)
